# revision 7
# baseline (speedup 1.0000x reference)
"""Trainium2 Bass kernel for nn_Attention_33011118637863.

ViT-style attention (SAM decomposed rel-pos bias) for B=8, N=1024 (32x32),
C=768, nh=12, hd=64.  Data-parallel over batch: one batch element per
NeuronCore, 8 cores, no collectives.

Per-core device program (all matmuls in fp32r = full-rate fp32):
  A2. qkT[c, n]   = (Wqk^T)^T-chunks @ x^T  (q rows 0:768, k rows 768:1536,
      k pre-scaled by 1/8 on host; bias fused into the PSUM->SBUF copy)
  A3. v[n, c]     = x @ Wv^T + bv (natural orientation, written into an
      augmented tile with a ones column -> softmax denominators for free)
  B.  rel_h/rel_w tables contracted with q per query-row/col (tiny matmuls,
      4-way tile_position packed)
  C.  St[j, i] = kT^T-scaled @ q + one-hot-expanded rel bias  (PSUM accum);
      exp via ACT; out^T[d, i] = sum_j v_aug[j, d|1] * exp[j, i]  (row 64 of
      the PSUM is the softmax denominator); normalize during PSUM evacuation
      (row-scaling commutes with the output projection).
  D.  out = (out^T)^T-chunks @ Wp^T + bp.

Host does only layout work: transposes, the rel_pos table gather, bias
replication, and the inverse transposes/scales on the way out (the k output
is descaled by exactly 8.0, a power of two, so it is bit-exact).
"""

import sys

for _p in ("/opt/trn_rl_repo",):
    if _p not in sys.path:
        sys.path.insert(0, _p)

import numpy as np

import concourse.bass as bass  # noqa: E402
import concourse.mybir as mybir  # noqa: E402
import concourse.tile as tile  # noqa: E402
from concourse import bacc  # noqa: E402
from concourse.bass_utils import run_bass_kernel_spmd  # noqa: E402

F32 = mybir.dt.float32
F32R = mybir.dt.float32r
AF = mybir.ActivationFunctionType
ADD = mybir.AluOpType.add
MUL = mybir.AluOpType.mult

B, N, C = 8, 1024, 768
NH, HD = 12, 64
NP = 6  # head pairs
CK = 6  # 128-wide chunks of C (contraction for qkv/proj)
NT = 8  # 128-wide token chunks
IT = 2  # 512-wide i-tiles


def _r(ap):
    return ap  # tensors feeding matmuls are declared float32r natively


def build_program():
    nc = bacc.Bacc("TRN2", target_bir_lowering=False, debug=False)

    xT_d = nc.dram_tensor("xT", [C, N], F32R, kind="ExternalInput")
    wqk_d = nc.dram_tensor("wqk", [C, 2 * C], F32R, kind="ExternalInput")
    bqk_d = nc.dram_tensor("bqk", [2 * C], F32, kind="ExternalInput")
    wv_d = nc.dram_tensor("wv", [C, C], F32R, kind="ExternalInput")
    bv_d = nc.dram_tensor("bv", [128, C], F32, kind="ExternalInput")
    wp_d = nc.dram_tensor("wp", [C, C], F32R, kind="ExternalInput")
    bp_d = nc.dram_tensor("bp", [128, C], F32, kind="ExternalInput")
    relh_d = nc.dram_tensor("relh", [128, 32, 32], F32R, kind="ExternalInput")
    relw_d = nc.dram_tensor("relw", [128, 32, 32], F32R, kind="ExternalInput")
    ebias_d = nc.dram_tensor("ebias", [128, 8, 128], F32R, kind="ExternalInput")
    ehead_d = nc.dram_tensor("ehead", [1, 64], F32R, kind="ExternalInput")
    ones_d = nc.dram_tensor("ones", [128, 96], F32R, kind="ExternalInput")

    out_d = nc.dram_tensor("out", [N, C], F32, kind="ExternalOutput")
    kT_d = nc.dram_tensor("kT", [C, N], F32R, kind="ExternalOutput")
    v_d = nc.dram_tensor("v", [N, C], F32R, kind="ExternalOutput")

    with tile.TileContext(nc) as tc:
        _emit(nc, tc, locals())
    nc.compile()
    return nc


def _emit(nc, tc, t):
    from contextlib import ExitStack

    ctx = ExitStack()
    const = ctx.enter_context(tc.tile_pool(name="const", bufs=1))
    persist = ctx.enter_context(tc.tile_pool(name="persist", bufs=1))
    ps = ctx.enter_context(tc.tile_pool(name="ps", bufs=8, space="PSUM"))

    # ---- constants ----
    bqk_sb = const.tile([128, 12], F32, tag="bqk")
    nc.sync.dma_start(bqk_sb[:], t["bqk_d"].ap().rearrange("(o p) -> p o", p=128))
    bv_sb = const.tile([128, C], F32, tag="bv")
    nc.sync.dma_start(bv_sb[:], t["bv_d"].ap())
    bp_sb = const.tile([128, C], F32, tag="bp")
    nc.sync.dma_start(bp_sb[:], t["bp_d"].ap())
    relh_sb = const.tile([128, 32, 32], F32R, tag="relh")
    nc.sync.dma_start(relh_sb[:], t["relh_d"].ap())
    relw_sb = const.tile([128, 32, 32], F32R, tag="relw")
    nc.sync.dma_start(relw_sb[:], t["relw_d"].ap())
    ebias_sb = const.tile([128, 8, 128], F32R, tag="ebias")
    nc.sync.dma_start(ebias_sb[:], t["ebias_d"].ap())
    ehead_sb = const.tile([1, 64], F32R, tag="ehead")
    nc.sync.dma_start(ehead_sb[:], t["ehead_d"].ap())

    # ---- persistent tiles ----
    qkT = [persist.tile([128, N], F32R, tag=f"qk{oc}", name=f"qk{oc}") for oc in range(12)]
    v_aug = persist.tile([128, NT, NH, HD + 1], F32R, tag="vaug")
    outT = persist.tile([128, CK, N], F32R, tag="outT")
    nc.sync.dma_start(
        v_aug[:, :, :, HD],
        t["ones_d"].ap().rearrange("p (a b) -> p a b", b=NH),
    )

    # ================= stage A: load x/w, QKV =================
    with tc.tile_pool(name="stageA", bufs=1) as pa:
        xT_t, wqk_t, wv_t = [], [], []
        for o in range(CK):
            xt = pa.tile([128, N], F32R, tag=f"x{o}")
            nc.sync.dma_start(xt[:], t["xT_d"].ap()[o * 128 : (o + 1) * 128, :])
            xT_t.append(xt)
            wq = pa.tile([128, 2 * C], F32R, tag=f"wqk{o}")
            nc.sync.dma_start(wq[:], t["wqk_d"].ap()[o * 128 : (o + 1) * 128, :])
            wqk_t.append(wq)
            wv = pa.tile([128, C], F32R, tag=f"wv{o}")
            nc.sync.dma_start(wv[:], t["wv_d"].ap()[o * 128 : (o + 1) * 128, :])
            wv_t.append(wv)

        # A2: transposed q (oc 0..5) and scaled k (oc 6..11)
        for oc in range(12):
            for it in range(IT):
                p = ps.tile([128, 512], F32, tag="ps")
                for o in range(CK):
                    nc.tensor.matmul(
                        p[:],
                        _r(wqk_t[o][:, oc * 128 : (oc + 1) * 128]),
                        _r(xT_t[o][:, it * 512 : (it + 1) * 512]),
                        start=(o == 0),
                        stop=(o == CK - 1),
                    )
                nc.scalar.activation(
                    qkT[oc][:, it * 512 : (it + 1) * 512],
                    p[:],
                    AF.Identity,
                    bias=bqk_sb[:, oc : oc + 1],
                )
            if oc >= 6:
                nc.sync.dma_start(
                    t["kT_d"].ap()[(oc - 6) * 128 : (oc - 5) * 128, :], qkT[oc][:]
                )

        # A3: natural-orientation v into the augmented tile (+ ones col)
        for nt in range(NT):
            for vc in range(2):
                p = ps.tile([128, 384], F32, tag="ps")
                for o in range(CK):
                    nc.tensor.matmul(
                        p[:],
                        _r(xT_t[o][:, nt * 128 : (nt + 1) * 128]),
                        _r(wv_t[o][:, vc * 384 : (vc + 1) * 384]),
                        start=(o == 0),
                        stop=(o == CK - 1),
                    )
                nc.vector.tensor_tensor(
                    v_aug[:, nt, vc * 6 : (vc + 1) * 6, 0:HD],
                    p[:].rearrange("p (h d) -> p h d", d=HD),
                    bv_sb[:, vc * 384 : (vc + 1) * 384].rearrange(
                        "p (h d) -> p h d", d=HD
                    ),
                    ADD,
                )
            nc.sync.dma_start(
                t["v_d"].ap()[nt * 128 : (nt + 1) * 128, :],
                v_aug[:, nt, :, 0:HD],
            )

    # ================= stages B+C per head pair =================
    with (
        tc.tile_pool(name="relp", bufs=2) as relp,
        tc.tile_pool(name="expp", bufs=2) as expp,
        tc.tile_pool(name="rp", bufs=2) as rpool,
    ):
        for pr in range(NP):
            q = qkT[pr]
            k = qkT[6 + pr]
            qw = q.rearrange("p (h w) -> p w h", w=32)

            # ---- B: rel tables (rows: 0:32 relh_h0, 32:64 relw_h0,
            #         64:96 relh_h1, 96:128 relw_h1; relw halves stored
            #         w-major in PSUM, permuted into i-order on copy) ----
            rel = relp.tile([128, N], F32R, tag="rel")
            relw_v = rel.rearrange("p (h w) -> p w h", w=32)
            for hf in range(2):
                pRH0 = ps.tile([32, 512], F32, tag="ps", name=f"pRH0_{pr}_{hf}")
                pRH1 = ps.tile([32, 512], F32, tag="ps", name=f"pRH1_{pr}_{hf}")
                pRW0 = ps.tile([32, 512], F32, tag="ps", name=f"pRW0_{pr}_{hf}")
                pRW1 = ps.tile([32, 512], F32, tag="ps", name=f"pRW1_{pr}_{hf}")
                for sl in range(16):
                    s = hf * 16 + sl
                    col = sl * 32
                    nc.tensor.matmul(
                        pRH0[:, col : col + 32],
                        relh_sb[0:64, s, :], q[0:64, s * 32 : (s + 1) * 32],
                        start=True, stop=True, tile_position=(0, 0),
                    )
                    nc.tensor.matmul(
                        pRH1[:, col : col + 32],
                        relh_sb[64:128, s, :], q[64:128, s * 32 : (s + 1) * 32],
                        start=True, stop=True, tile_position=(64, 0),
                    )
                    nc.tensor.matmul(
                        pRW0[:, col : col + 32],
                        relw_sb[0:64, s, :], qw[0:64, s, :],
                        start=True, stop=True, tile_position=(0, 0),
                    )
                    nc.tensor.matmul(
                        pRW1[:, col : col + 32],
                        relw_sb[64:128, s, :], qw[64:128, s, :],
                        start=True, stop=True, tile_position=(64, 0),
                    )
                isl = slice(hf * 512, (hf + 1) * 512)
                wsl = slice(hf * 16, (hf + 1) * 16)
                nc.scalar.copy(rel[0:32, isl], pRH0[:])
                nc.scalar.copy(rel[64:96, isl], pRH1[:])
                nc.vector.tensor_copy(relw_v[32:64, wsl, :], pRW0[:])
                nc.vector.tensor_copy(relw_v[96:128, wsl, :], pRW1[:])

            # ---- C: scores + bias -> exp -> attn@v (+denominator) ----
            for it in range(IT):
                isl = slice(it * 512, (it + 1) * 512)
                ex = expp.tile([128, NT, 2 * 512], F32R, tag="exp")
                for jc in range(NT):
                    jsl = slice(jc * 128, (jc + 1) * 128)
                    st0 = ps.tile([128, 512], F32, tag="ps")
                    st1 = ps.tile([128, 512], F32, tag="ps")
                    nc.tensor.matmul(
                        st0[:], _r(k[0:64, jsl]), _r(q[0:64, isl]),
                        start=True, stop=False, tile_position=(0, 0),
                    )
                    nc.tensor.matmul(
                        st0[:], _r(ebias_sb[0:64, jc, :]), _r(rel[0:64, isl]),
                        start=False, stop=True, tile_position=(0, 0),
                    )
                    nc.tensor.matmul(
                        st1[:], _r(k[64:128, jsl]), _r(q[64:128, isl]),
                        start=True, stop=False, tile_position=(64, 0),
                    )
                    nc.tensor.matmul(
                        st1[:], _r(ebias_sb[64:128, jc, :]), _r(rel[64:128, isl]),
                        start=False, stop=True, tile_position=(64, 0),
                    )
                    nc.scalar.activation(ex[:, jc, 0:512], st0[:], AF.Exp)
                    nc.scalar.activation(ex[:, jc, 512:1024], st1[:], AF.Exp)

                po0 = ps.tile([128, 512], F32, tag="ps")
                po1 = ps.tile([128, 512], F32, tag="ps")
                for jc in range(NT):
                    nc.tensor.matmul(
                        po0[0:65, :], _r(v_aug[:, jc, 2 * pr, :]),
                        _r(ex[:, jc, 0:512]),
                        start=(jc == 0), stop=(jc == NT - 1),
                    )
                for jc in range(NT):
                    nc.tensor.matmul(
                        po1[0:65, :], _r(v_aug[:, jc, 2 * pr + 1, :]),
                        _r(ex[:, jc, 512:1024]),
                        start=(jc == 0), stop=(jc == NT - 1),
                    )
                rp0 = rpool.tile([1, 512], F32R, tag="rp0")
                rp1 = rpool.tile([1, 512], F32R, tag="rp1")
                with nc.allow_low_precision(reason="fp32r rounding of softmax recip"):
                    nc.vector.reciprocal(rp0[:], po0[64:65, :])
                    nc.vector.reciprocal(rp1[:], po1[64:65, :])
                prr0 = ps.tile([64, 512], F32, tag="ps")
                prr1 = ps.tile([64, 512], F32, tag="ps")
                nc.tensor.matmul(prr0[:], _r(ehead_sb[:]), _r(rp0[:]),
                                 start=True, stop=True, tile_position=(0, 0))
                nc.tensor.matmul(prr1[:], _r(ehead_sb[:]), _r(rp1[:]),
                                 start=True, stop=True, tile_position=(0, 0))
                rf = rpool.tile([128, 512], F32, tag="rfull")
                nc.vector.tensor_copy(rf[0:64, :], prr0[:])
                nc.vector.tensor_copy(rf[64:128, :], prr1[:])
                nc.vector.tensor_tensor(
                    outT[0:64, pr, isl], po0[0:64, :], rf[0:64, :], MUL
                )
                nc.vector.tensor_tensor(
                    outT[64:128, pr, isl], po1[0:64, :], rf[64:128, :], MUL
                )

    # ================= stage D: output projection =================
    with tc.tile_pool(name="stageD", bufs=1) as pd, tc.tile_pool(
        name="osb", bufs=2
    ) as osb:
        wp_t = []
        for o in range(CK):
            w = pd.tile([128, C], F32R, tag=f"wp{o}")
            nc.sync.dma_start(w[:], t["wp_d"].ap()[o * 128 : (o + 1) * 128, :])
            wp_t.append(w)
        for nt in range(NT):
            o_sb = osb.tile([128, C], F32, tag="osb")
            for pc in range(2):
                p = ps.tile([128, 384], F32, tag="ps")
                for o in range(CK):
                    nc.tensor.matmul(
                        p[:],
                        _r(outT[:, o, nt * 128 : (nt + 1) * 128]),
                        _r(wp_t[o][:, pc * 384 : (pc + 1) * 384]),
                        start=(o == 0),
                        stop=(o == CK - 1),
                    )
                nc.vector.tensor_tensor(
                    o_sb[:, pc * 384 : (pc + 1) * 384],
                    p[:],
                    bp_sb[:, pc * 384 : (pc + 1) * 384],
                    ADD,
                )
            nc.sync.dma_start(t["out_d"].ap()[nt * 128 : (nt + 1) * 128, :], o_sb[:])

    ctx.close()


# ---------------- host side ----------------

_NC_CACHE = {}


def _get_program():
    if "nc" not in _NC_CACHE:
        _NC_CACHE["nc"] = build_program()
    return _NC_CACHE["nc"]


def _host_inputs(x, qkv_w, qkv_b, proj_w, proj_b, rel_pos_h, rel_pos_w):
    f = np.float32
    x = np.asarray(x, f)
    qkv_w = np.asarray(qkv_w, f)
    qkv_b = np.asarray(qkv_b, f)
    proj_w = np.asarray(proj_w, f)
    proj_b = np.asarray(proj_b, f)
    rel_pos_h = np.asarray(rel_pos_h, f)
    rel_pos_w = np.asarray(rel_pos_w, f)

    scale = np.float32(1.0 / 8.0)  # hd ** -0.5, exact power of two
    wqk = np.ascontiguousarray(qkv_w[: 2 * C].T).copy()
    wqk[:, C:] *= scale
    bqk = qkv_b[: 2 * C].copy()
    bqk[C:] *= scale
    wv = np.ascontiguousarray(qkv_w[2 * C :].T)
    bv = np.tile(qkv_b[2 * C :][None, :], (128, 1)).astype(f)
    wp = np.ascontiguousarray(proj_w.T)
    bp = np.tile(proj_b[None, :], (128, 1)).astype(f)

    idx = np.arange(32)[:, None] - np.arange(32)[None, :] + 31
    Rh = rel_pos_h[idx]  # (32 s, 32 m, 64 c)
    Rw = rel_pos_w[idx]
    relh = np.zeros((128, 32, 32), f)
    relh[0:64] = Rh.transpose(2, 0, 1)
    relh[64:128] = relh[0:64]
    relw = np.zeros((128, 32, 32), f)
    relw[0:64] = Rw.transpose(2, 0, 1)
    relw[64:128] = relw[0:64]

    ebias = np.zeros((128, 8, 128), f)
    m = np.arange(32)
    j = np.arange(128)
    for jc in range(8):
        eh = (jc * 4 + j[None, :] // 32 == m[:, None]).astype(f)
        ew = (j[None, :] % 32 == m[:, None]).astype(f)
        ebias[0:32, jc] = eh
        ebias[32:64, jc] = ew
        ebias[64:96, jc] = eh
        ebias[96:128, jc] = ew
    ehead = np.ones((1, 64), f)
    ones = np.ones((128, 96), f)

    shared = {
        "wqk": wqk, "bqk": bqk, "wv": wv, "bv": bv, "wp": wp, "bp": bp,
        "relh": relh, "relw": relw, "ebias": ebias, "ehead": ehead,
        "ones": ones,
    }
    in_maps = [
        {**shared, "xT": np.ascontiguousarray(x[b].T)} for b in range(B)
    ]
    return in_maps


def _unshard(results):
    out = np.stack([results[b]["out"] for b in range(B)])
    k = np.stack(
        [
            results[b]["kT"].reshape(NH, HD, N).transpose(0, 2, 1) * np.float32(8.0)
            for b in range(B)
        ]
    ).reshape(B * NH, N, HD)
    v = np.stack(
        [results[b]["v"].reshape(N, NH, HD).transpose(1, 0, 2) for b in range(B)]
    ).reshape(B * NH, N, HD)
    pre_kv = np.stack([k, v])
    return out, pre_kv


def kernel(x, qkv_w, qkv_b, proj_w, proj_b, rel_pos_h, rel_pos_w, H=32, W=32, **kw):
    assert int(H) == 32 and int(W) == 32
    in_maps = _host_inputs(x, qkv_w, qkv_b, proj_w, proj_b, rel_pos_h, rel_pos_w)
    nc = _get_program()
    res = run_bass_kernel_spmd(nc, in_maps, list(range(B)))
    return _unshard(res.results)
